# revision 50
# baseline (speedup 1.0000x reference)
"""CapsuleLayer kernel for Trainium2 (8 NeuronCores, Bass/Tile).

Math: reference einsum("bhwf,fcd->bhwd", x, Wc) sums over BOTH f and c,
so it collapses to a single matmul:
    W_eff[f, d] = sum_c capsules.reshape(F, C, D)[f, c, d]
    out = x.reshape(-1, F) @ W_eff            # (100352, 256) @ (256, 16)

Distribution: data-parallel over flattened positions (batch*H*W), 12544
positions per core; the small capsule weight is replicated. Each core
receives its x shard pre-transposed to (F, PPC) so the contraction dim f
sits on SBUF partitions (the tensor engine contracts over partitions);
the core emits outT (16, PPC) which the host transposes back.

Modes (host-side dtype of the streamed x shard + PE matmul dtype):
  'fp8'  - x quantized to fp8 e3m4 (1-byte stream), weights kept in
           fp16 (mixed-dtype matmul, verified exact on HW), fp16 output
           store. Kernel is memory-bound so the 1-byte stream is ~2x
           the fp16 mode. rel err ~1.35e-2 (gate 2e-2): e3m4 keeps 4
           mantissa bits and randn x never leaves its normal range.
  'fp16' - x/W rounded to fp16, 2-byte stream, rel err ~2.9e-4
  'f32r' - float32r matmul (1 cycle/row), full 4-byte stream
  'fp32' - exact float32 matmul (4 PE cycles/row), full 4-byte stream

fp8 layout: the contraction index is split f = 2p + a (p = SBUF
partition, a = 0,1) so the capsule weight (f-major, 160 floats per f)
loads as one contiguous 1280B line per partition - the baseline's
(k p) split needed 2x640B strided descriptors per line and crawled at
23 GB/s on the critical sync ring, stalling the x stream ~2.4us.
x chunk DMAs see the same 2-segments-per-partition shape either way.

Measured fp8 (per-core NTFF exec, 8 cores concurrent): ~25.5us, down
from the 34.5us fp16 baseline. Anatomy (trace ts, first-useful marker
~2.6us): 0-6.5 fixed NEFF/Tile engine rendezvous (gated by GPSIMD Q7
boot); 6.5-9.0 SDMA engine wake ramp (engines 6-15 sleep until ~8.5);
9.0-17.9 the 3.37MB fp8 stream at ~380-400 GB/s; trailing matmuls +
PSUM drains to ~20.7; two parallel output stores to ~23; HBM write
ack + sem clears + exit barrier to ~28. Things measured NOT to work:
mid-stream stores (their HBM-ack stalls SDMA engines and delays the
trailing load sems by up to 2.7us), >1 store per ring (data+ack
serialize per engine), ACT-split PSUM drains (three separate
regressions), 16-partition output stores (reach 4 SDMA engines, ~90
GB/s), 448-pos tail-chunk loads (448B segments below the 512B SDMA
line-rate floor).
"""

import numpy as np
import ml_dtypes

import concourse.bass as bass  # noqa: F401  (engine types referenced via nc)
import concourse.tile as tile
from concourse import bacc, mybir
from concourse.bass_utils import run_bass_kernel_spmd

N_CORES = 8
B, H, W, F = 32, 56, 56, 256
NUM_CAPS, CAP_DIM = 10, 16
POS = B * H * W            # 100352
PPC = POS // N_CORES       # 12544 positions per core
SUB = 448                  # matmul moving free dim (<=512 fp32 PSUM bank)
NT = 4 * SUB               # 1792 positions per chunk (4 col-tiled strips)
KC = F // 128              # 2 contraction chunks of 128

MODE = "fp8"               # default; see module docstring

_MM_DT = {
    "fp32": mybir.dt.float32,
    "f32r": mybir.dt.float32r,
    "fp16": mybir.dt.float16,
    "fp8": mybir.dt.float8e3,
}
_NP_DT = {
    "fp32": np.float32,
    "f32r": np.float32,
    "fp16": np.float16,
    "fp8": ml_dtypes.float8_e3m4,
}

_cache = {}


def _build_fp8():
    """fp8 e3m4 x-stream, fp16 weights, fp16 output.

    vs the fp16 mode: (1) caps loads contiguously (f = 2p + a layout,
    one 1280B line per partition) so it clears the sync ring in ~0.5us;
    (2) a burst of dummy matmuls on a zeroed tile right after the
    barrier keeps the PE busy through one HAM SHORT window, so the real
    matmuls run at 2.4 GHz instead of the 1.2 GHz cold clock (measured
    cold: chunk matmuls lag the stream by ~3.7us); (3) loads are 7
    uniform 1792-position chunks (1792B segments - a 448-pos tail
    chunk's 448B segments fall under the 512B SDMA line-rate floor and
    measurably crawl); (4) chunks 0-5 store on the SWDGE path while the
    rings stream input; only the last chunk's 4 tiny strip-stores ride
    the rings at the end, keeping the completion-lag chain short.
    """
    NBIG = 7               # 7*1792 = 12544, no tail chunks
    nc = bacc.Bacc(
        None,
        target_bir_lowering=False,
        debug=False,
        enable_asserts=False,
        num_devices=N_CORES,
    )

    xT = nc.dram_tensor("xT", [F, PPC], mybir.dt.float8e3, kind="ExternalInput")
    caps = nc.dram_tensor(
        "caps", [F, NUM_CAPS * CAP_DIM], mybir.dt.float32, kind="ExternalInput"
    )
    # Raw PSUM-drain layout, strip-packed: row 32g+16k+d (d<16) holds
    # strip s=2g+k of each chunk - two strips share one 32-row PE column
    # group via zero-padded weights ([Weff|0] writes rows 0-15 of the
    # group, [0|Weff] accumulates rows 16-31), so the output tile is
    # DENSE: 64 partitions, no garbage rows. Dense [64, N] stores halve
    # the post-stream store bytes vs the naive [128, N]-with-garbage
    # layout (a [16, PPC] layout is worse still: 4 SDMA engines, ~90
    # GB/s). Host: out[off_c + (2g+k)*SUB + n, d] = outR[32g+16k+d, ...].
    # 3 big chunks of 3584 positions (8 strips -> strip 2g+k packed into
    # row 32g+16k+d of all FOUR column groups: fully dense [128, SUB]
    # output column) + 1 trailing 1792 chunk (2 groups, rows 0-63).
    NCOL = 4
    outR = nc.dram_tensor("outR", [128, NCOL * SUB], mybir.dt.float16,
                          kind="ExternalOutput")

    with tile.TileContext(nc) as tc:
        with (
            tc.tile_pool(name="const", bufs=1) as cpool,
            tc.tile_pool(name="xinb", bufs=NBIG) as xpool_b,
            tc.tile_pool(name="psumb", bufs=4, space="PSUM") as pspool_b,
        ):
            # ---- PE warm-up: ~4.3us of dummy matmuls -------------------
            # Depends on nothing (operand deliberately uninitialized, the
            # result is never read) -> runs right at barrier-release,
            # during the otherwise-dead DMA ramp. One fully-busy HAM
            # SHORT window flips the PE clock gate 4/8 -> 8/8; the streak
            # then continues into the real matmuls so it never re-arms.
            # Without this the whole kernel runs matmuls at the 1.2 GHz
            # cold clock and compute lags the stream by ~3us at the end.
            zt = cpool.tile([128, 512], mybir.dt.float8e3, tag="zt")
            nc.vector.memset(zt[:], 0)
            ps_w = pspool_b.tile([128, 512], mybir.dt.float32, tag="ps")
            for i in range(10):
                nc.tensor.matmul(ps_w[:, :], zt[:, 0:128], zt[:],
                                 start=(i == 0), stop=(i == 9))

            # ---- W_eff = sum over capsules, f = 2p + a layout ------------
            # contiguous load (partition p <- caps rows 2p, 2p+1 = 1280B)
            # first on the sync ring (~0.5us), while scalar leads with x.
            ct = cpool.tile([128, KC * NUM_CAPS * CAP_DIM], mybir.dt.float32,
                            tag="caps")
            nc.sync.dma_start(ct[:], caps.rearrange("(p a) c -> p (a c)", p=128))
            w32 = cpool.tile([128, KC, CAP_DIM], mybir.dt.float32, tag="w32")
            for a in range(KC):
                sl = slice(a * NUM_CAPS * CAP_DIM, (a + 1) * NUM_CAPS * CAP_DIM)
                nc.vector.reduce_sum(
                    w32[:, a, :],
                    ct[:, sl].rearrange("p (c d) -> p d c", c=NUM_CAPS),
                    axis=mybir.AxisListType.X,
                )
            # zero-padded weight pair: wz[:, a, 0, :] = [Weff | 0] and
            # wz[:, a, 1, :] = [0 | Weff], so two strips can accumulate
            # into disjoint 16-row halves of one 32-row PE column group.
            # Written fully (memset + 2 copies) before any LDWEIGHTS.
            wz = cpool.tile([128, KC, 2, 2 * CAP_DIM], mybir.dt.float16,
                            tag="wz")
            nc.vector.memset(wz[:], 0)
            nc.vector.tensor_copy(wz[:, :, 0, 0:CAP_DIM], w32[:])
            nc.vector.tensor_copy(wz[:, :, 1, CAP_DIM:2 * CAP_DIM], w32[:])

            # ---- streaming loads on both HWDGE rings ---------------------
            # Deliberately asymmetric: sync gets caps+c1+c3 and empties
            # early (it then runs the big store); scalar gets the rest
            # INCLUDING the trailing chunks, so ring-FIFO staggers their
            # completion - earlier trailing chunks' matmuls+copies overlap
            # the last load's tail (SDMA engines round-robin the two
            # rings at packet granularity, so ring-alternating chunks
            # finish together). The final chunk is split 1344+448: the
            # mini-chunk's completion sem closes sooner after last-byte
            # (sem spread scales with transfer size) and its PSUM drain
            # is a quarter-size copy, shortening the endgame chain.
            # Chunk j covers positions [off_j, off_j + sz_j). Fewer,
            # bigger transfers sustain line rate with fewer sem/transfer
            # boundaries; the 1792 tail keeps the last transfer small so
            # its completion sem closes quickly after last-byte.
            sizes = [2 * NT] * 3 + [NT]
            offs = [sum(sizes[:k]) for k in range(len(sizes))]
            assert offs[-1] + sizes[-1] == PPC
            xT_v = xT.rearrange("(p a) n -> p a n", a=KC)  # [128, KC, PPC]
            xts = []
            for j, (o, sz) in enumerate(zip(offs, sizes)):
                cols = slice(o, o + sz)
                xt = xpool_b.tile([128, KC, sz], mybir.dt.float8e3,
                                  tag=f"xt{sz}")
                ring = nc.sync if j == 1 else nc.scalar
                ring.dma_start(xt[:], xT_v[:, :, cols])
                xts.append(xt)

            # resident output buffer: one dense fp16 column per chunk
            # (strip 2g+k on partitions 32g+16k..32g+16k+15); the last
            # (1792) chunk fills only rows 0-63.
            ob = cpool.tile([128, NCOL, SUB], mybir.dt.float16, tag="ob")

            for j, (o, sz) in enumerate(zip(offs, sizes)):
                xt = xts[j]
                ngrp = sz // NT * 2   # 4 groups for 3584, 2 for 1792
                # col-tiled groups, 2 strips packed per group, all into
                # ONE PSUM bank; a single dense copy drains the chunk.
                ps = pspool_b.tile([128, 512], mybir.dt.float32, tag="ps")
                for g in range(ngrp):
                    for k in range(2):
                        s = 2 * g + k
                        sl = slice(s * SUB, (s + 1) * SUB)
                        for a in range(KC):
                            nc.tensor.matmul(
                                ps[32 * g: 32 * g + 32, 0:SUB],
                                wz[:, a, k, :],
                                xt[:, a, sl],
                                start=(k == 0 and a == 0),
                                stop=(k == 1 and a == KC - 1),
                                tile_position=(0, 32 * g),
                            )
                rows = 32 * ngrp
                nc.vector.tensor_copy(ob[0:rows, j, :], ps[0:rows, 0:SUB])

            # ---- stores ------------------------------------------------
            # NO store may overlap the input stream: a store's sem-inc
            # descriptor waits for the HBM write ack (~1-2us), stalling
            # that SDMA engine's queue - mid-stream stores were measured
            # to delay the trailing load sems by up to 2.7us; likewise
            # two sequential stores on one ring serialize data+ack per
            # engine, so each ring carries exactly ONE store. Merged
            # [128, 6*SUB] store for chunks 0-5 on sync (dispatches when
            # c5's copy lands, just after the stream ends); the last
            # chunk's [128, SUB] store on scalar in parallel.
            nc.sync.dma_start(outR[:, 0:(NCOL - 1) * SUB],
                              ob[:, 0:NCOL - 1, :])
            nc.scalar.dma_start(outR[0:64, (NCOL - 1) * SUB:NCOL * SUB],
                                ob[0:64, NCOL - 1, :])

    nc.compile()
    return nc


def _build(mode: str):
    if mode == "fp8":
        return _build_fp8()
    nc = bacc.Bacc(
        None,
        target_bir_lowering=False,
        debug=False,
        enable_asserts=False,
        num_devices=N_CORES,
    )
    mm_dt = _MM_DT[mode]
    SUBF = 448
    NTF = 4 * SUBF
    NBIG = 6

    xT = nc.dram_tensor("xT", [F, PPC], mm_dt, kind="ExternalInput")
    caps = nc.dram_tensor(
        "caps", [F, NUM_CAPS * CAP_DIM], mybir.dt.float32, kind="ExternalInput"
    )
    outT = nc.dram_tensor("outT", [CAP_DIM, PPC], mybir.dt.float32, kind="ExternalOutput")

    with tile.TileContext(nc) as tc:
        with (
            tc.tile_pool(name="const", bufs=1) as cpool,
            tc.tile_pool(name="xinb", bufs=NBIG) as xpool_b,
            tc.tile_pool(name="xins", bufs=4) as xpool_s,
            tc.tile_pool(name="psumb", bufs=4, space="PSUM") as pspool_b,
            tc.tile_pool(name="psums", bufs=4, space="PSUM") as pspool_s,
        ):
            # ---- W_eff = sum over capsules of the (F, C*D) weight --------
            ct = cpool.tile([128, KC, NUM_CAPS * CAP_DIM], mybir.dt.float32, tag="caps")
            nc.sync.dma_start(ct[:], caps.rearrange("(k p) c -> p k c", p=128))
            w32 = cpool.tile([128, KC, CAP_DIM], mybir.dt.float32, tag="w32")
            for k in range(KC):
                nc.vector.reduce_sum(
                    w32[:, k, :],
                    ct[:, k, :].rearrange("p (c d) -> p d c", c=NUM_CAPS),
                    axis=mybir.AxisListType.X,
                )
            weff = cpool.tile([128, KC, CAP_DIM], mm_dt, tag="weff")
            nc.vector.tensor_copy(weff[:], w32[:])

            chunks = []
            off = 0
            for sz in [NTF] * NBIG + [SUBF] * 4:
                chunks.append((off, sz))
                off += sz
            assert off == PPC

            xT_v = xT.rearrange("(k p) n -> p k n", k=KC)  # [128, KC, PPC]
            xts = []
            for j, (o, sz) in enumerate(chunks):
                cols = slice(o, o + sz)
                pool = xpool_b if sz == NTF else xpool_s
                xt = pool.tile([128, KC, sz], mm_dt, tag=f"xt{sz}")
                ring = nc.sync if j % 2 == 0 else nc.scalar
                ring.dma_start(xt[:], xT_v[:, :, cols])
                xts.append(xt)

            HALF_A = 4
            ob_a = cpool.tile([128, HALF_A, SUBF], mybir.dt.float32, tag="oba")
            ob_b = cpool.tile([128, NBIG - HALF_A, SUBF], mybir.dt.float32, tag="obb")
            ob_t = []
            for s in range(4):
                obt = cpool.tile([CAP_DIM, SUBF], mybir.dt.float32, tag=f"obt{s}")
                ob_t.append(obt)

            def ob_slot(col):
                if col < HALF_A:
                    return ob_a, col
                return ob_b, col - HALF_A

            for j, (o, sz) in enumerate(chunks):
                xt = xts[j]
                if sz == NTF:
                    ps = pspool_b.tile([128, 512], mybir.dt.float32, tag="psb")
                    for s in range(4):
                        sl = slice(s * SUBF, (s + 1) * SUBF)
                        for k in range(KC):
                            nc.tensor.matmul(
                                ps[32 * s: 32 * s + CAP_DIM, 0:SUBF],
                                weff[:, k, :],
                                xt[:, k, sl],
                                start=(k == 0),
                                stop=(k == KC - 1),
                                tile_position=(0, 32 * s),
                            )
                    ob, col = ob_slot(j)
                    nc.vector.tensor_copy(ob[:, col, :], ps[:, 0:SUBF])
                else:
                    s = j - NBIG
                    ps = pspool_s.tile([CAP_DIM, 512], mybir.dt.float32, tag="pss")
                    for k in range(KC):
                        nc.tensor.matmul(
                            ps[:, 0:SUBF],
                            weff[:, k, :],
                            xt[:, k, :],
                            start=(k == 0),
                            stop=(k == KC - 1),
                        )
                    nc.vector.tensor_copy(ob_t[s][:], ps[:, 0:SUBF])

            outT_s = outT.rearrange("d (c s n) -> d s c n", s=4, n=SUBF)
            for s in range(4):
                nc.gpsimd.dma_start(
                    outT_s[:, s, 0:HALF_A, :],
                    ob_a[32 * s: 32 * s + CAP_DIM, :, :],
                )
            for s in range(4):
                ring = nc.sync if s % 2 == 0 else nc.scalar
                ring.dma_start(
                    outT_s[:, s, HALF_A:NBIG, :],
                    ob_b[32 * s: 32 * s + CAP_DIM, :, :],
                )
            for s in range(4):
                ring = nc.sync if s % 2 == 0 else nc.scalar
                ring.dma_start(outT_s[:, s, NBIG, :], ob_t[s][:])

    nc.compile()
    return nc


def _get_nc(mode: str):
    if mode not in _cache:
        _cache[mode] = _build(mode)
    return _cache[mode]


def run(x, capsules, trace=False, trace_cores=None, mode=None):
    """Shard, execute on 8 cores, gather. Returns (out, BassKernelResults)."""
    if mode is None:
        mode = MODE
    nc = _get_nc(mode)

    x = np.asarray(x, dtype=np.float32)
    capsules = np.asarray(capsules, dtype=np.float32)
    xf = x.reshape(POS, F).astype(_NP_DT[mode], copy=False)
    caps2 = np.ascontiguousarray(capsules.reshape(F, NUM_CAPS * CAP_DIM))
    xT_full = xf.T  # view; per-core slices are copied once during input concat

    in_maps = [
        {"xT": xT_full[:, c * PPC: (c + 1) * PPC], "caps": caps2}
        for c in range(N_CORES)
    ]
    res = run_bass_kernel_spmd(
        nc,
        in_maps,
        core_ids=list(range(N_CORES)),
        trace=trace,
        trace_cores=trace_cores,
    )
    out = np.empty((POS, CAP_DIM), dtype=np.float32)
    for c in range(N_CORES):
        if mode == "fp8":
            # outR[32g+16k+d, cc*SUB+n] -> out[cc*3584 + (2g+k)*SUB+n, d]
            # (big chunks use 4 groups; the last 1792 chunk uses 2).
            raw = res.results[c]["outR"]               # [128, 4*SUB] fp16
            dst = out[c * PPC: (c + 1) * PPC]
            npk = 3 * 2 * NT
            v = raw[:, 0:3 * SUB].reshape(4, 2, CAP_DIM, 3, SUB)
            dst[0:npk] = (
                v.transpose(3, 0, 1, 4, 2).reshape(npk, CAP_DIM)
                .astype(np.float32)
            )
            w = raw[0:64, 3 * SUB:].reshape(2, 2, CAP_DIM, SUB)
            dst[npk:] = (
                w.transpose(0, 1, 3, 2).reshape(NT, CAP_DIM)
                .astype(np.float32)
            )
        else:
            out[c * PPC: (c + 1) * PPC] = res.results[c]["outT"].T.astype(np.float32)
    return out.reshape(B, H, W, CAP_DIM), res


def kernel(x, capsules):
    out, _ = run(x, capsules)
    return out


# revision 51
# speedup vs baseline: 1.0658x; 1.0658x over previous
"""CapsuleLayer kernel for Trainium2 (8 NeuronCores, Bass/Tile).

Math: reference einsum("bhwf,fcd->bhwd", x, Wc) sums over BOTH f and c,
so it collapses to a single matmul:
    W_eff[f, d] = sum_c capsules.reshape(F, C, D)[f, c, d]
    out = x.reshape(-1, F) @ W_eff            # (100352, 256) @ (256, 16)

Distribution: data-parallel over flattened positions (batch*H*W), 12544
positions per core; the small capsule weight is replicated. Each core
receives its x shard pre-transposed to (F, PPC) so the contraction dim f
sits on SBUF partitions (the tensor engine contracts over partitions);
the core emits outT (16, PPC) which the host transposes back.

Modes (host-side dtype of the streamed x shard + PE matmul dtype):
  'fp8'  - x quantized to fp8 e3m4 (1-byte stream), weights kept in
           fp16 (mixed-dtype matmul, verified exact on HW), fp16 output
           store. Kernel is memory-bound so the 1-byte stream is ~2x
           the fp16 mode. rel err ~1.35e-2 (gate 2e-2): e3m4 keeps 4
           mantissa bits and randn x never leaves its normal range.
  'fp16' - x/W rounded to fp16, 2-byte stream, rel err ~2.9e-4
  'f32r' - float32r matmul (1 cycle/row), full 4-byte stream
  'fp32' - exact float32 matmul (4 PE cycles/row), full 4-byte stream

fp8 layout: the contraction index is split f = 2p + a (p = SBUF
partition, a = 0,1) so the capsule weight (f-major, 160 floats per f)
loads as one contiguous 1280B line per partition - the baseline's
(k p) split needed 2x640B strided descriptors per line and crawled at
23 GB/s on the critical sync ring, stalling the x stream ~2.4us.
x chunk DMAs see the same 2-segments-per-partition shape either way.

Measured fp8 (per-core NTFF exec, 8 cores concurrent): ~25.5us, down
from the 34.5us fp16 baseline. Anatomy (trace ts, first-useful marker
~2.6us): 0-6.5 fixed NEFF/Tile engine rendezvous (gated by GPSIMD Q7
boot); 6.5-9.0 SDMA engine wake ramp (engines 6-15 sleep until ~8.5);
9.0-17.9 the 3.37MB fp8 stream at ~380-400 GB/s; trailing matmuls +
PSUM drains to ~20.7; two parallel output stores to ~23; HBM write
ack + sem clears + exit barrier to ~28. Things measured NOT to work:
mid-stream stores (their HBM-ack stalls SDMA engines and delays the
trailing load sems by up to 2.7us), >1 store per ring (data+ack
serialize per engine), ACT-split PSUM drains (three separate
regressions), 16-partition output stores (reach 4 SDMA engines, ~90
GB/s), 448-pos tail-chunk loads (448B segments below the 512B SDMA
line-rate floor).
"""

import numpy as np
import ml_dtypes

import concourse.bass as bass  # noqa: F401  (engine types referenced via nc)
import concourse.tile as tile
from concourse import bacc, mybir
from concourse.bass_utils import run_bass_kernel_spmd

N_CORES = 8
B, H, W, F = 32, 56, 56, 256
NUM_CAPS, CAP_DIM = 10, 16
POS = B * H * W            # 100352
PPC = POS // N_CORES       # 12544 positions per core
SUB = 448                  # matmul moving free dim (<=512 fp32 PSUM bank)
NT = 4 * SUB               # 1792 positions per chunk (4 col-tiled strips)
KC = F // 128              # 2 contraction chunks of 128

MODE = "fp8"               # default; see module docstring

_MM_DT = {
    "fp32": mybir.dt.float32,
    "f32r": mybir.dt.float32r,
    "fp16": mybir.dt.float16,
    "fp8": mybir.dt.float8e3,
}
_NP_DT = {
    "fp32": np.float32,
    "f32r": np.float32,
    "fp16": np.float16,
    "fp8": ml_dtypes.float8_e3m4,
}

_cache = {}


def _build_fp8():
    """fp8 e3m4 x-stream, fp16 weights, fp16 output.

    vs the fp16 mode: (1) caps loads contiguously (f = 2p + a layout,
    one 1280B line per partition) so it clears the sync ring in ~0.5us;
    (2) a burst of dummy matmuls on a zeroed tile right after the
    barrier keeps the PE busy through one HAM SHORT window, so the real
    matmuls run at 2.4 GHz instead of the 1.2 GHz cold clock (measured
    cold: chunk matmuls lag the stream by ~3.7us); (3) loads are 7
    uniform 1792-position chunks (1792B segments - a 448-pos tail
    chunk's 448B segments fall under the 512B SDMA line-rate floor and
    measurably crawl); (4) chunks 0-5 store on the SWDGE path while the
    rings stream input; only the last chunk's 4 tiny strip-stores ride
    the rings at the end, keeping the completion-lag chain short.
    """
    NBIG = 7               # 7*1792 = 12544, no tail chunks
    nc = bacc.Bacc(
        None,
        target_bir_lowering=False,
        debug=False,
        enable_asserts=False,
        num_devices=N_CORES,
    )

    xT = nc.dram_tensor("xT", [F, PPC], mybir.dt.float8e3, kind="ExternalInput")
    caps = nc.dram_tensor(
        "caps", [F, NUM_CAPS * CAP_DIM], mybir.dt.float32, kind="ExternalInput"
    )
    # Raw PSUM-drain layout, strip-packed: row 32g+16k+d (d<16) holds
    # strip s=2g+k of each chunk - two strips share one 32-row PE column
    # group via zero-padded weights ([Weff|0] writes rows 0-15 of the
    # group, [0|Weff] accumulates rows 16-31), so the output tile is
    # DENSE: 64 partitions, no garbage rows. Dense [64, N] stores halve
    # the post-stream store bytes vs the naive [128, N]-with-garbage
    # layout (a [16, PPC] layout is worse still: 4 SDMA engines, ~90
    # GB/s). Host: out[off_c + (2g+k)*SUB + n, d] = outR[32g+16k+d, ...].
    NCOL = PPC // NT       # 7 uniform chunks
    outR = nc.dram_tensor("outR", [64, NCOL * SUB], mybir.dt.float16,
                          kind="ExternalOutput")

    with tile.TileContext(nc) as tc:
        with (
            tc.tile_pool(name="const", bufs=1) as cpool,
            tc.tile_pool(name="xinb", bufs=NBIG) as xpool_b,
            tc.tile_pool(name="psumb", bufs=4, space="PSUM") as pspool_b,
        ):
            # ---- PE warm-up: ~4.3us of dummy matmuls -------------------
            # Depends on nothing (operand deliberately uninitialized, the
            # result is never read) -> runs right at barrier-release,
            # during the otherwise-dead DMA ramp. One fully-busy HAM
            # SHORT window flips the PE clock gate 4/8 -> 8/8; the streak
            # then continues into the real matmuls so it never re-arms.
            # Without this the whole kernel runs matmuls at the 1.2 GHz
            # cold clock and compute lags the stream by ~3us at the end.
            zt = cpool.tile([128, 512], mybir.dt.float8e3, tag="zt")
            nc.vector.memset(zt[:], 0)
            ps_w = pspool_b.tile([128, 512], mybir.dt.float32, tag="ps")
            for i in range(10):
                nc.tensor.matmul(ps_w[:, :], zt[:, 0:128], zt[:],
                                 start=(i == 0), stop=(i == 9))

            # ---- W_eff = sum over capsules, f = 2p + a layout ------------
            # contiguous load (partition p <- caps rows 2p, 2p+1 = 1280B)
            # first on the sync ring (~0.5us), while scalar leads with x.
            ct = cpool.tile([128, KC * NUM_CAPS * CAP_DIM], mybir.dt.float32,
                            tag="caps")
            nc.sync.dma_start(ct[:], caps.rearrange("(p a) c -> p (a c)", p=128))
            w32 = cpool.tile([128, KC, CAP_DIM], mybir.dt.float32, tag="w32")
            for a in range(KC):
                sl = slice(a * NUM_CAPS * CAP_DIM, (a + 1) * NUM_CAPS * CAP_DIM)
                nc.vector.reduce_sum(
                    w32[:, a, :],
                    ct[:, sl].rearrange("p (c d) -> p d c", c=NUM_CAPS),
                    axis=mybir.AxisListType.X,
                )
            # zero-padded weight pair: wz[:, a, 0, :] = [Weff | 0] and
            # wz[:, a, 1, :] = [0 | Weff], so two strips can accumulate
            # into disjoint 16-row halves of one 32-row PE column group.
            # Written fully (memset + 2 copies) before any LDWEIGHTS.
            wz = cpool.tile([128, KC, 2, 2 * CAP_DIM], mybir.dt.float16,
                            tag="wz")
            nc.vector.memset(wz[:], 0)
            nc.vector.tensor_copy(wz[:, :, 0, 0:CAP_DIM], w32[:])
            nc.vector.tensor_copy(wz[:, :, 1, CAP_DIM:2 * CAP_DIM], w32[:])

            # ---- streaming loads on both HWDGE rings ---------------------
            # Deliberately asymmetric: sync gets caps+c1+c3 and empties
            # early (it then runs the big store); scalar gets the rest
            # INCLUDING the trailing chunks, so ring-FIFO staggers their
            # completion - earlier trailing chunks' matmuls+copies overlap
            # the last load's tail (SDMA engines round-robin the two
            # rings at packet granularity, so ring-alternating chunks
            # finish together). The final chunk is split 1344+448: the
            # mini-chunk's completion sem closes sooner after last-byte
            # (sem spread scales with transfer size) and its PSUM drain
            # is a quarter-size copy, shortening the endgame chain.
            # Chunk j covers positions [off_j, off_j + sz_j).
            sizes = [NT] * NBIG
            offs = [sum(sizes[:k]) for k in range(len(sizes))]
            assert offs[-1] + sizes[-1] == PPC
            xT_v = xT.rearrange("(p a) n -> p a n", a=KC)  # [128, KC, PPC]
            xts = []
            for j, (o, sz) in enumerate(zip(offs, sizes)):
                cols = slice(o, o + sz)
                xt = xpool_b.tile([128, KC, sz], mybir.dt.float8e3,
                                  tag=f"xt{sz}")
                ring = nc.sync if j in (1, 3) else nc.scalar
                ring.dma_start(xt[:], xT_v[:, :, cols])
                xts.append(xt)

            # resident output buffer: one dense [64, SUB] fp16 column per
            # chunk (strip 2g+k on partitions 32g+16k..32g+16k+15).
            ob = cpool.tile([64, NCOL, SUB], mybir.dt.float16, tag="ob")

            for j, (o, sz) in enumerate(zip(offs, sizes)):
                xt = xts[j]
                # 2 col-tiled groups per chunk, 2 strips packed per group
                # into ONE PSUM bank; a single dense [64, SUB] copy
                # drains the whole chunk.
                ps = pspool_b.tile([128, 512], mybir.dt.float32, tag="ps")
                for g in range(2):
                    for k in range(2):
                        s = 2 * g + k
                        sl = slice(s * SUB, (s + 1) * SUB)
                        for a in range(KC):
                            nc.tensor.matmul(
                                ps[32 * g: 32 * g + 32, 0:SUB],
                                wz[:, a, k, :],
                                xt[:, a, sl],
                                start=(k == 0 and a == 0),
                                stop=(k == 1 and a == KC - 1),
                                tile_position=(0, 32 * g),
                            )
                nc.vector.tensor_copy(ob[:, j, :], ps[0:64, 0:SUB])

            # ---- stores ------------------------------------------------
            # NO store may overlap the input stream: a store's sem-inc
            # descriptor waits for the HBM write ack (~1-2us), stalling
            # that SDMA engine's queue - mid-stream stores were measured
            # to delay the trailing load sems by up to 2.7us; likewise
            # two sequential stores on one ring serialize data+ack per
            # engine, so each ring carries exactly ONE store. Merged
            # [128, 6*SUB] store for chunks 0-5 on sync (dispatches when
            # c5's copy lands, just after the stream ends); the last
            # chunk's [128, SUB] store on scalar in parallel.
            nc.sync.dma_start(outR[:, 0:(NCOL - 1) * SUB],
                              ob[:, 0:NCOL - 1, :])
            nc.scalar.dma_start(outR[:, (NCOL - 1) * SUB:NCOL * SUB],
                                ob[:, NCOL - 1:NCOL, :])

    nc.compile()
    return nc


def _build(mode: str):
    if mode == "fp8":
        return _build_fp8()
    nc = bacc.Bacc(
        None,
        target_bir_lowering=False,
        debug=False,
        enable_asserts=False,
        num_devices=N_CORES,
    )
    mm_dt = _MM_DT[mode]
    SUBF = 448
    NTF = 4 * SUBF
    NBIG = 6

    xT = nc.dram_tensor("xT", [F, PPC], mm_dt, kind="ExternalInput")
    caps = nc.dram_tensor(
        "caps", [F, NUM_CAPS * CAP_DIM], mybir.dt.float32, kind="ExternalInput"
    )
    outT = nc.dram_tensor("outT", [CAP_DIM, PPC], mybir.dt.float32, kind="ExternalOutput")

    with tile.TileContext(nc) as tc:
        with (
            tc.tile_pool(name="const", bufs=1) as cpool,
            tc.tile_pool(name="xinb", bufs=NBIG) as xpool_b,
            tc.tile_pool(name="xins", bufs=4) as xpool_s,
            tc.tile_pool(name="psumb", bufs=4, space="PSUM") as pspool_b,
            tc.tile_pool(name="psums", bufs=4, space="PSUM") as pspool_s,
        ):
            # ---- W_eff = sum over capsules of the (F, C*D) weight --------
            ct = cpool.tile([128, KC, NUM_CAPS * CAP_DIM], mybir.dt.float32, tag="caps")
            nc.sync.dma_start(ct[:], caps.rearrange("(k p) c -> p k c", p=128))
            w32 = cpool.tile([128, KC, CAP_DIM], mybir.dt.float32, tag="w32")
            for k in range(KC):
                nc.vector.reduce_sum(
                    w32[:, k, :],
                    ct[:, k, :].rearrange("p (c d) -> p d c", c=NUM_CAPS),
                    axis=mybir.AxisListType.X,
                )
            weff = cpool.tile([128, KC, CAP_DIM], mm_dt, tag="weff")
            nc.vector.tensor_copy(weff[:], w32[:])

            chunks = []
            off = 0
            for sz in [NTF] * NBIG + [SUBF] * 4:
                chunks.append((off, sz))
                off += sz
            assert off == PPC

            xT_v = xT.rearrange("(k p) n -> p k n", k=KC)  # [128, KC, PPC]
            xts = []
            for j, (o, sz) in enumerate(chunks):
                cols = slice(o, o + sz)
                pool = xpool_b if sz == NTF else xpool_s
                xt = pool.tile([128, KC, sz], mm_dt, tag=f"xt{sz}")
                ring = nc.sync if j % 2 == 0 else nc.scalar
                ring.dma_start(xt[:], xT_v[:, :, cols])
                xts.append(xt)

            HALF_A = 4
            ob_a = cpool.tile([128, HALF_A, SUBF], mybir.dt.float32, tag="oba")
            ob_b = cpool.tile([128, NBIG - HALF_A, SUBF], mybir.dt.float32, tag="obb")
            ob_t = []
            for s in range(4):
                obt = cpool.tile([CAP_DIM, SUBF], mybir.dt.float32, tag=f"obt{s}")
                ob_t.append(obt)

            def ob_slot(col):
                if col < HALF_A:
                    return ob_a, col
                return ob_b, col - HALF_A

            for j, (o, sz) in enumerate(chunks):
                xt = xts[j]
                if sz == NTF:
                    ps = pspool_b.tile([128, 512], mybir.dt.float32, tag="psb")
                    for s in range(4):
                        sl = slice(s * SUBF, (s + 1) * SUBF)
                        for k in range(KC):
                            nc.tensor.matmul(
                                ps[32 * s: 32 * s + CAP_DIM, 0:SUBF],
                                weff[:, k, :],
                                xt[:, k, sl],
                                start=(k == 0),
                                stop=(k == KC - 1),
                                tile_position=(0, 32 * s),
                            )
                    ob, col = ob_slot(j)
                    nc.vector.tensor_copy(ob[:, col, :], ps[:, 0:SUBF])
                else:
                    s = j - NBIG
                    ps = pspool_s.tile([CAP_DIM, 512], mybir.dt.float32, tag="pss")
                    for k in range(KC):
                        nc.tensor.matmul(
                            ps[:, 0:SUBF],
                            weff[:, k, :],
                            xt[:, k, :],
                            start=(k == 0),
                            stop=(k == KC - 1),
                        )
                    nc.vector.tensor_copy(ob_t[s][:], ps[:, 0:SUBF])

            outT_s = outT.rearrange("d (c s n) -> d s c n", s=4, n=SUBF)
            for s in range(4):
                nc.gpsimd.dma_start(
                    outT_s[:, s, 0:HALF_A, :],
                    ob_a[32 * s: 32 * s + CAP_DIM, :, :],
                )
            for s in range(4):
                ring = nc.sync if s % 2 == 0 else nc.scalar
                ring.dma_start(
                    outT_s[:, s, HALF_A:NBIG, :],
                    ob_b[32 * s: 32 * s + CAP_DIM, :, :],
                )
            for s in range(4):
                ring = nc.sync if s % 2 == 0 else nc.scalar
                ring.dma_start(outT_s[:, s, NBIG, :], ob_t[s][:])

    nc.compile()
    return nc


def _get_nc(mode: str):
    if mode not in _cache:
        _cache[mode] = _build(mode)
    return _cache[mode]


def run(x, capsules, trace=False, trace_cores=None, mode=None):
    """Shard, execute on 8 cores, gather. Returns (out, BassKernelResults)."""
    if mode is None:
        mode = MODE
    nc = _get_nc(mode)

    x = np.asarray(x, dtype=np.float32)
    capsules = np.asarray(capsules, dtype=np.float32)
    xf = x.reshape(POS, F).astype(_NP_DT[mode], copy=False)
    caps2 = np.ascontiguousarray(capsules.reshape(F, NUM_CAPS * CAP_DIM))
    xT_full = xf.T  # view; per-core slices are copied once during input concat

    in_maps = [
        {"xT": xT_full[:, c * PPC: (c + 1) * PPC], "caps": caps2}
        for c in range(N_CORES)
    ]
    res = run_bass_kernel_spmd(
        nc,
        in_maps,
        core_ids=list(range(N_CORES)),
        trace=trace,
        trace_cores=trace_cores,
    )
    out = np.empty((POS, CAP_DIM), dtype=np.float32)
    for c in range(N_CORES):
        if mode == "fp8":
            # outR[32g+16k+d, cc*SUB+n] -> out[cc*NT + (2g+k)*SUB + n, d]
            raw = res.results[c]["outR"]               # [64, NCOL*SUB] fp16
            ncol = PPC // NT
            v = raw.reshape(2, 2, CAP_DIM, ncol, SUB)  # [g, k, d, cc, n]
            out[c * PPC: (c + 1) * PPC] = (
                v.transpose(3, 0, 1, 4, 2).reshape(PPC, CAP_DIM)
                .astype(np.float32)
            )
        else:
            out[c * PPC: (c + 1) * PPC] = res.results[c]["outT"].T.astype(np.float32)
    return out.reshape(B, H, W, CAP_DIM), res


def kernel(x, capsules):
    out, _ = run(x, capsules)
    return out
